# revision 14
# baseline (speedup 1.0000x reference)
"""Block-sparse DSD matmul  y = x @ W^T  on 8 TRN2 NeuronCores.

x: [2048, 4096] f32, W given as 2048 sparse 32x32 blocks at (rows, cols)
block coordinates in a 128x128 block grid. y: [2048, 4096] f32.

Strategy (batch-parallel SPMD, identical program on 8 cores):
  - Shard batch 8 ways (256 rows/core); the sparse structure is identical
    on every core so one SPMD program works with per-core x shards.
  - bf16 x and W (f32 PSUM accumulation) — one PE pass per matmul and
    half the HBM traffic; y returned as bf16 and widened on host.
  - Compute y^T tiles on-chip: for block (r, c):
        y^T[32r:32r+32, :] += W_blk @ x^T[32c:32c+32, :]
    As a PE matmul: out = lhsT.T @ rhs with lhsT = W_blk^T (stationary,
    32x32), rhs = x^T chunk [32, 256], tile_position picks the 32x32 PE
    subarray: row group a = c%4 (SBUF strip), col group b = output strip.
  - Rows are sorted by nnz count and grouped 4-at-a-time (similar counts
    together) into 32 groups; each group accumulates directly into its
    own PSUM bank ([128, 256] tile, bank-granular pool) via per-strip
    has_written chains — no cross-lane tree-add, one evacuation copy.
  - Matmuls are emitted in "waves" of <=4 with pairwise-distinct col
    groups (and lanes), letting up to 4 subarray matmul streams overlap.
    A greedy scheduler packs slots into waves across a sliding window of
    active groups and prefers early x chunks in early waves.
"""

import numpy as np

# toggles used by test.py only; harness uses defaults
_RUN = {"trace": False, "trace_cores": [0], "last": None}

B, K, OUT, BLK, NNZ = 2048, 4096, 4096, 32, 2048
NCORES = 8
BC = B // NCORES          # 256 batch rows per core
NT = K // 128             # 32 x^T partition-tiles
NRB = OUT // BLK          # 128 row blocks
NG = NRB // 4             # 32 groups of 4 row blocks
NXCH = 8                  # x^T DMA chunks
XC = NT // NXCH           # x tiles per chunk
NWCH = 8                  # weight DMA chunks


def _build_schedule(w, rows, cols):
    """Sorted-count grouping + greedy wave schedule + packed weights.

    Region (g, b) accumulates row order[4g+b] on PSUM bank g's strip b,
    always from PE subarray (32b, 32b) — a PSUM region must only ever be
    written from one tile_position row group (HW constraint).  The rhs
    x^T strip (c%4) is independent of the tile position; waves prefer
    pairwise-distinct strips to share the 128-partition moving port.
    """
    import ml_dtypes

    cnt = np.bincount(rows, minlength=NRB)
    order = np.argsort(-cnt, kind="stable")
    place = {int(r): (k // 4, k % 4) for k, r in enumerate(order)}

    # slots[g][b] = [t, s, n]: blocks of row order[4g+b]; s = rhs strip.
    # n == -1 is a zero-weight dummy for otherwise-untouched PSUM regions.
    slots = [[[] for _ in range(4)] for _ in range(NG)]
    for n in range(NNZ):
        g, b = place[int(rows[n])]
        slots[g][b].append([int(cols[n]) // 4, int(cols[n]) % 4, n])
    for r in range(NRB):
        if cnt[r] == 0:
            g, b = place[r]
            slots[g][b].append([0, 0, -1])
    for g in range(NG):
        for b in range(4):
            slots[g][b].sort(key=lambda s: s[0])

    n_gb = np.array([[len(slots[g][b]) for b in range(4)] for g in range(NG)])

    MAX_ACTIVE = 7
    remaining = [[list(l) for l in gl] for gl in slots]
    done_g = [all(not l for l in gl) for gl in remaining]
    seen_gb = np.zeros((NG, 4), dtype=np.int64)
    waves = []
    scheduled = 0
    total = int(n_gb.sum())
    while scheduled < total:
        wv = len(waves)
        maxchunk = 1 + wv // 24
        wave = []
        acts = [g for g in range(NG) if not done_g[g]][:MAX_ACTIVE]
        for b in range(4):
            pick = None
            for g in acts:
                for i, (t, s, n) in enumerate(remaining[g][b]):
                    if t // XC >= maxchunk:
                        continue
                    pick = (g, i, t, s, n)
                    break
                if pick:
                    break
            if pick is None:
                continue
            g, i, t, s, n = pick
            remaining[g][b].pop(i)
            st = seen_gb[g, b] == 0
            sp = seen_gb[g, b] == n_gb[g, b] - 1
            seen_gb[g, b] += 1
            wave.append((s, g, b, t, n, bool(st), bool(sp)))
            scheduled += 1
            if all(not l for l in remaining[g]):
                done_g[g] = True
        waves.append(wave)

    W = len(waves)
    WCH = -(-W // NWCH)
    Wpad = WCH * NWCH
    wpk = np.zeros((128, Wpad * BLK), dtype=ml_dtypes.bfloat16)
    for wv, wave in enumerate(waves):
        for s, g, b, t, n, st, sp in wave:
            if n >= 0:
                wpk[32 * b:32 * b + 32, wv * BLK:(wv + 1) * BLK] = \
                    np.ascontiguousarray(w[n].T).astype(ml_dtypes.bfloat16)

    # wave index after which each group is fully accumulated
    evac = [-1] * NG
    for wv, wave in enumerate(waves):
        for s, g, b, t, n, st, sp in wave:
            evac[g] = max(evac[g], wv)
    return waves, WCH, Wpad, wpk, order, evac


def _build_module(waves, WCH, Wpad, evac):
    import concourse.bacc as bacc
    import concourse.tile as tile
    import concourse.mybir as mybir
    from contextlib import ExitStack

    f32 = mybir.dt.float32
    bf16 = mybir.dt.bfloat16
    nc = bacc.Bacc()
    xt_d = nc.declare_dram_parameter("xt", [128, NT * BC], bf16, isOutput=False)
    wp_d = nc.declare_dram_parameter("wpk", [128, Wpad * BLK], bf16,
                                     isOutput=False)
    yt_d = nc.declare_dram_parameter("yt", [128, NG, BC], f32, isOutput=True)

    with tile.TileContext(nc) as tc, ExitStack() as ctx:
        xp = ctx.enter_context(tc.tile_pool(name="x", bufs=1))
        wpool = ctx.enter_context(tc.tile_pool(name="w", bufs=3))
        pp = ctx.enter_context(tc.tile_pool(name="ps", bufs=8, space="PSUM"))
        yp = ctx.enter_context(tc.tile_pool(name="y", bufs=4))

        wtiles = {}

        def load_w(c):
            wsb = wpool.tile([128, WCH * BLK], bf16, tag="w", name=f"w{c}")
            nc.sync.dma_start(
                wsb[:], wp_d[:, c * WCH * BLK:(c + 1) * WCH * BLK])
            wtiles[c] = wsb

        # xrot[k][ci]: partition-rotated x^T chunk: partition p holds
        # original partition (p - 32k) mod 128, so x^T strip s appears at
        # strip (s + k) % 4.  Lets any column feed any PE row group while
        # keeping weights/fmap/tile_position partition-aligned.
        xrot = [[] for _ in range(4)]

        def load_x(ci):
            lo, hi = ci * XC * BC, (ci + 1) * XC * BC
            for k in range(4):
                xc = xp.tile([128, XC * BC], bf16, tag=f"x{k}_{ci}",
                             name=f"x{k}_{ci}")
                if k == 0:
                    nc.sync.dma_start(xc[:], xt_d[:, lo:hi])
                else:
                    p = 32 * k
                    nc.sync.dma_start(xc[p:128, :], xt_d[0:128 - p, lo:hi])
                    nc.sync.dma_start(xc[0:p, :], xt_d[128 - p:128, lo:hi])
                xrot[k].append(xc)

        load_w(0)
        for ci in range(NXCH):
            load_x(ci)
        load_w(1)

        ptiles = {}
        for wv, wave in enumerate(waves):
            c = wv // WCH
            if wv == c * WCH and c + 2 < NWCH and (c + 2) * WCH < len(waves):
                load_w(c + 2)
            wsb = wtiles[c]
            for s, g, b, t, n, st, sp in wave:
                if g not in ptiles:
                    ptiles[g] = pp.tile([128, BC], f32, tag="ps",
                                        name=f"ps{g}")
                nc.tensor.matmul(
                    ptiles[g][32 * b:32 * b + 32, :],
                    lhsT=wsb[32 * b:32 * b + 32,
                             (wv - c * WCH) * BLK:(wv - c * WCH + 1) * BLK],
                    rhs=xrot[(b - s) % 4][t // XC][
                        32 * b:32 * b + 32,
                        (t % XC) * BC:(t % XC + 1) * BC],
                    start=st, stop=sp, skip_group_check=True,
                    tile_position=(32 * b, 32 * b),
                )
            for g in range(NG):
                if evac[g] == wv:
                    ps = ptiles.pop(g)
                    ysb = yp.tile([128, BC], f32, tag="y", name=f"y{g}")
                    nc.scalar.copy(ysb[:], ps[:])
                    nc.sync.dma_start(yt_d[:, g, :], ysb[:])

    nc.compile()
    return nc


def kernel(x, w, rows, cols, out_blocks=None):
    import ml_dtypes
    from concourse.bass_utils import run_bass_kernel_spmd

    x = np.asarray(x, dtype=np.float32)
    w = np.asarray(w, dtype=np.float32)
    rows = np.asarray(rows).astype(np.int64)
    cols = np.asarray(cols).astype(np.int64)

    waves, WCH, Wpad, wpk, order, evac = _build_schedule(w, rows, cols)
    nc = _build_module(waves, WCH, Wpad, evac)

    # x^T, per-core partition-major: xarr[core, p, t*BC + j] = x[BC*core + j, 128*t + p]
    xarr = np.ascontiguousarray(
        x.reshape(NCORES, BC, NT, 128).transpose(0, 3, 2, 1)
    ).reshape(NCORES, 128, NT * BC).astype(ml_dtypes.bfloat16)

    in_maps = [{"xt": xarr[i], "wpk": wpk} for i in range(NCORES)]
    res = run_bass_kernel_spmd(
        nc, in_maps, list(range(NCORES)),
        trace=_RUN["trace"], trace_cores=_RUN["trace_cores"],
    )
    _RUN["last"] = res

    # feature index of flat position (k=4g+b, i): 32*order[k] + i
    feat = (32 * order[:, None] + np.arange(32)[None, :]).ravel()

    y = np.empty((B, OUT), dtype=np.float32)
    for i in range(NCORES):
        ytp = np.asarray(res.results[i]["yt"]).astype(np.float32)
        # [128, NG, 256]: partition 32b+i, group g, batch j -> k=4g+b
        v = ytp.reshape(4, 32, NG, BC).transpose(2, 0, 1, 3)
        yT = np.empty((OUT, BC), dtype=np.float32)
        yT[feat] = v.reshape(OUT, BC)
        y[BC * i:BC * (i + 1), :] = yT.T
    return y


# revision 15
# speedup vs baseline: 1.1117x; 1.1117x over previous
"""Block-sparse DSD matmul  y = x @ W^T  on 8 TRN2 NeuronCores.

x: [2048, 4096] f32, W given as 2048 sparse 32x32 blocks at (rows, cols)
block coordinates in a 128x128 block grid. y: [2048, 4096] f32.

Strategy (batch-parallel SPMD, identical program on 8 cores):
  - Shard batch 8 ways (256 rows/core); the sparse structure is identical
    on every core so one SPMD program works with per-core x shards.
  - bf16 x and W (f32 PSUM accumulation) — one PE pass per matmul and
    half the HBM traffic; y returned as bf16 and widened on host.
  - Compute y^T tiles on-chip: for block (r, c):
        y^T[32r:32r+32, :] += W_blk @ x^T[32c:32c+32, :]
    As a PE matmul: out = lhsT.T @ rhs with lhsT = W_blk^T (stationary,
    32x32), rhs = x^T chunk [32, 256], tile_position picks the 32x32 PE
    subarray: row group a = c%4 (SBUF strip), col group b = output strip.
  - Rows are sorted by nnz count and grouped 4-at-a-time (similar counts
    together) into 32 groups; each group accumulates directly into its
    own PSUM bank ([128, 256] tile, bank-granular pool) via per-strip
    has_written chains — no cross-lane tree-add, one evacuation copy.
  - Matmuls are emitted in "waves" of <=4 with pairwise-distinct col
    groups (and lanes), letting up to 4 subarray matmul streams overlap.
    A greedy scheduler packs slots into waves across a sliding window of
    active groups and prefers early x chunks in early waves.
"""

import numpy as np

# toggles used by test.py only; harness uses defaults
_RUN = {"trace": False, "trace_cores": [0], "last": None}

B, K, OUT, BLK, NNZ = 2048, 4096, 4096, 32, 2048
NCORES = 8
BC = B // NCORES          # 256 batch rows per core
NT = K // 128             # 32 x^T partition-tiles
NRB = OUT // BLK          # 128 row blocks
NG = NRB // 4             # 32 groups of 4 row blocks
NXCH = 8                  # x^T DMA chunks
XC = NT // NXCH           # x tiles per chunk
NWCH = 8                  # weight DMA chunks


def _build_schedule(w, rows, cols):
    """Sorted-count grouping + greedy wave schedule + packed weights.

    Region (g, b) accumulates row order[4g+b] on PSUM bank g's strip b,
    always from PE subarray (32b, 32b) — a PSUM region must only ever be
    written from one tile_position row group (HW constraint).  The rhs
    x^T strip (c%4) is independent of the tile position; waves prefer
    pairwise-distinct strips to share the 128-partition moving port.
    """
    import ml_dtypes

    cnt = np.bincount(rows, minlength=NRB)
    order = np.argsort(-cnt, kind="stable")
    place = {int(r): (k // 4, k % 4) for k, r in enumerate(order)}

    # slots[g][b] = [t, s, n]: blocks of row order[4g+b]; s = rhs strip.
    # n == -1 is a zero-weight dummy for otherwise-untouched PSUM regions.
    slots = [[[] for _ in range(4)] for _ in range(NG)]
    for n in range(NNZ):
        g, b = place[int(rows[n])]
        slots[g][b].append([int(cols[n]) // 4, int(cols[n]) % 4, n])
    for r in range(NRB):
        if cnt[r] == 0:
            g, b = place[r]
            slots[g][b].append([0, 0, -1])
    for g in range(NG):
        for b in range(4):
            slots[g][b].sort(key=lambda s: s[0])

    n_gb = np.array([[len(slots[g][b]) for b in range(4)] for g in range(NG)])

    MAX_ACTIVE = 8
    remaining = [[list(l) for l in gl] for gl in slots]
    done_g = [all(not l for l in gl) for gl in remaining]
    seen_gb = np.zeros((NG, 4), dtype=np.int64)
    waves = []
    scheduled = 0
    total = int(n_gb.sum())
    while scheduled < total:
        wv = len(waves)
        maxchunk = 1 + wv // 20
        used_a = set()
        wave = []
        acts = [g for g in range(NG) if not done_g[g]][:MAX_ACTIVE]
        for b in range(4):
            pick = None
            for g in acts:
                if (b + g) % 4 in used_a:
                    continue
                for i, (t, s, n) in enumerate(remaining[g][b]):
                    if t // XC >= maxchunk:
                        continue
                    pick = (g, i, t, s, n)
                    break
                if pick:
                    break
            if pick is None:
                continue
            g, i, t, s, n = pick
            remaining[g][b].pop(i)
            used_a.add((b + g) % 4)
            st = seen_gb[g, b] == 0
            sp = seen_gb[g, b] == n_gb[g, b] - 1
            seen_gb[g, b] += 1
            wave.append((s, g, b, t, n, bool(st), bool(sp)))
            scheduled += 1
            if all(not l for l in remaining[g]):
                done_g[g] = True
        waves.append(wave)

    W = len(waves)
    WCH = -(-W // NWCH)
    Wpad = WCH * NWCH
    wpk = np.zeros((128, Wpad * BLK), dtype=ml_dtypes.bfloat16)
    for wv, wave in enumerate(waves):
        for s, g, b, t, n, st, sp in wave:
            if n >= 0:
                A = (b + g) % 4
                wpk[32 * A:32 * A + 32, wv * BLK:(wv + 1) * BLK] = \
                    np.ascontiguousarray(w[n].T).astype(ml_dtypes.bfloat16)

    # wave index after which each group is fully accumulated
    evac = [-1] * NG
    for wv, wave in enumerate(waves):
        for s, g, b, t, n, st, sp in wave:
            evac[g] = max(evac[g], wv)
    return waves, WCH, Wpad, wpk, order, evac


def _build_module(waves, WCH, Wpad, evac):
    import concourse.bacc as bacc
    import concourse.tile as tile
    import concourse.mybir as mybir
    from contextlib import ExitStack

    f32 = mybir.dt.float32
    bf16 = mybir.dt.bfloat16
    nc = bacc.Bacc()
    xt_d = nc.declare_dram_parameter("xt", [128, 4, NT * BC], bf16,
                                     isOutput=False)
    wp_d = nc.declare_dram_parameter("wpk", [128, Wpad * BLK], bf16,
                                     isOutput=False)
    yt_d = nc.declare_dram_parameter("yt", [128, NG, BC], f32, isOutput=True)

    with tile.TileContext(nc) as tc, ExitStack() as ctx:
        xp = ctx.enter_context(tc.tile_pool(name="x", bufs=1))
        wpool = ctx.enter_context(tc.tile_pool(name="w", bufs=3))
        pp = ctx.enter_context(tc.tile_pool(name="ps", bufs=8, space="PSUM"))
        yp = ctx.enter_context(tc.tile_pool(name="y", bufs=4))

        wtiles = {}

        def load_w(c):
            wsb = wpool.tile([128, WCH * BLK], bf16, tag="w", name=f"w{c}")
            nc.sync.dma_start(
                wsb[:], wp_d[:, c * WCH * BLK:(c + 1) * WCH * BLK])
            wtiles[c] = wsb

        # xrot[k][ci]: x^T chunk pre-rotated on host so that original
        # strip s sits at partition strip (s + k) % 4.  Any column can feed
        # any PE row group while weights/fmap/tile_position stay aligned.
        # DMA issue costs ~600ns of sequencer time apiece, so issues are
        # split across the two HWDGE sequencers (sync + scalar).
        xrot = [[] for _ in range(4)]

        def load_x(ci):
            lo, hi = ci * XC * BC, (ci + 1) * XC * BC
            for k in range(4):
                xc = xp.tile([128, XC * BC], bf16, tag=f"x{k}_{ci}",
                             name=f"x{k}_{ci}")
                eng = nc.sync if k < 2 else nc.scalar
                eng.dma_start(xc[:], xt_d[:, k, lo:hi])
                xrot[k].append(xc)

        load_w(0)
        for ci in range(NXCH):
            load_x(ci)
        load_w(1)

        ptiles = {}
        for wv, wave in enumerate(waves):
            c = wv // WCH
            if wv == c * WCH and c + 2 < NWCH and (c + 2) * WCH < len(waves):
                load_w(c + 2)
            wsb = wtiles[c]
            for s, g, b, t, n, st, sp in wave:
                if g not in ptiles:
                    ptiles[g] = pp.tile([128, BC], f32, tag="ps",
                                        name=f"ps{g}")
                A = (b + g) % 4
                nc.tensor.matmul(
                    ptiles[g][32 * b:32 * b + 32, :],
                    lhsT=wsb[32 * A:32 * A + 32,
                             (wv - c * WCH) * BLK:(wv - c * WCH + 1) * BLK],
                    rhs=xrot[(A - s) % 4][t // XC][
                        32 * A:32 * A + 32,
                        (t % XC) * BC:(t % XC + 1) * BC],
                    start=st, stop=sp, skip_group_check=True,
                    tile_position=(32 * A, 32 * b),
                )
            for g in range(NG):
                if evac[g] == wv:
                    ps = ptiles.pop(g)
                    ysb = yp.tile([128, BC], f32, tag="y", name=f"y{g}")
                    nc.vector.tensor_copy(ysb[:], ps[:])
                    nc.scalar.dma_start(yt_d[:, g, :], ysb[:])

    nc.compile()
    return nc


def kernel(x, w, rows, cols, out_blocks=None):
    import ml_dtypes
    from concourse.bass_utils import run_bass_kernel_spmd

    x = np.asarray(x, dtype=np.float32)
    w = np.asarray(w, dtype=np.float32)
    rows = np.asarray(rows).astype(np.int64)
    cols = np.asarray(cols).astype(np.int64)

    waves, WCH, Wpad, wpk, order, evac = _build_schedule(w, rows, cols)
    nc = _build_module(waves, WCH, Wpad, evac)

    # x^T, per-core partition-major: xarr[core, p, t*BC + j] = x[BC*core + j, 128*t + p]
    xarr = np.ascontiguousarray(
        x.reshape(NCORES, BC, NT, 128).transpose(0, 3, 2, 1)
    ).reshape(NCORES, 128, NT * BC).astype(ml_dtypes.bfloat16)
    # 4 partition rotations: xrot[core, p, k, :] = xarr[core, (p - 32k) % 128, :]
    xrot = np.stack([np.roll(xarr, 32 * k, axis=1) for k in range(4)],
                    axis=2)

    in_maps = [{"xt": xrot[i], "wpk": wpk} for i in range(NCORES)]
    res = run_bass_kernel_spmd(
        nc, in_maps, list(range(NCORES)),
        trace=_RUN["trace"], trace_cores=_RUN["trace_cores"],
    )
    _RUN["last"] = res

    # feature index of flat position (k=4g+b, i): 32*order[k] + i
    feat = (32 * order[:, None] + np.arange(32)[None, :]).ravel()

    y = np.empty((B, OUT), dtype=np.float32)
    for i in range(NCORES):
        ytp = np.asarray(res.results[i]["yt"]).astype(np.float32)
        # [128, NG, 256]: partition 32b+i, group g, batch j -> k=4g+b
        v = ytp.reshape(4, 32, NG, BC).transpose(2, 0, 1, 3)
        yT = np.empty((OUT, BC), dtype=np.float32)
        yT[feat] = v.reshape(OUT, BC)
        y[BC * i:BC * (i + 1), :] = yT.T
    return y


# revision 16
# speedup vs baseline: 1.1812x; 1.0625x over previous
"""Block-sparse DSD matmul  y = x @ W^T  on 8 TRN2 NeuronCores.

x: [2048, 4096] f32, W given as 2048 sparse 32x32 blocks at (rows, cols)
block coordinates in a 128x128 block grid. y: [2048, 4096] f32.

Strategy (batch-parallel SPMD, identical program on 8 cores):
  - Shard batch 8 ways (256 rows/core); the sparse structure is identical
    on every core so one SPMD program works with per-core x shards.
  - bf16 x and W (f32 PSUM accumulation) — one PE pass per matmul and
    half the HBM traffic; y returned as bf16 and widened on host.
  - Compute y^T tiles on-chip: for block (r, c):
        y^T[32r:32r+32, :] += W_blk @ x^T[32c:32c+32, :]
    As a PE matmul: out = lhsT.T @ rhs with lhsT = W_blk^T (stationary,
    32x32), rhs = x^T chunk [32, 256], tile_position picks the 32x32 PE
    subarray: row group a = c%4 (SBUF strip), col group b = output strip.
  - Rows are sorted by nnz count and grouped 4-at-a-time (similar counts
    together) into 32 groups; each group accumulates directly into its
    own PSUM bank ([128, 256] tile, bank-granular pool) via per-strip
    has_written chains — no cross-lane tree-add, one evacuation copy.
  - Matmuls are emitted in "waves" of <=4 with pairwise-distinct col
    groups (and lanes), letting up to 4 subarray matmul streams overlap.
    A greedy scheduler packs slots into waves across a sliding window of
    active groups and prefers early x chunks in early waves.
"""

import numpy as np

# toggles used by test.py only; harness uses defaults
_RUN = {"trace": False, "trace_cores": [0], "last": None}

B, K, OUT, BLK, NNZ = 2048, 4096, 4096, 32, 2048
NCORES = 8
BC = B // NCORES          # 256 batch rows per core
NT = K // 128             # 32 x^T partition-tiles
NRB = OUT // BLK          # 128 row blocks
NG = NRB // 4             # 32 groups of 4 row blocks
NXCH = 8                  # x^T DMA chunks
XC = NT // NXCH           # x tiles per chunk
NWCH = 8                  # weight DMA chunks


def _build_schedule(w, rows, cols):
    """Sorted-count grouping + greedy wave schedule + packed weights.

    Region (g, b) accumulates row order[4g+b] on PSUM bank g's strip b,
    always from PE subarray (32b, 32b) — a PSUM region must only ever be
    written from one tile_position row group (HW constraint).  The rhs
    x^T strip (c%4) is independent of the tile position; waves prefer
    pairwise-distinct strips to share the 128-partition moving port.
    """
    import ml_dtypes

    cnt = np.bincount(rows, minlength=NRB)
    order = np.argsort(-cnt, kind="stable")
    place = {int(r): (k // 4, k % 4) for k, r in enumerate(order)}

    # slots[g][b] = [t, s, n]: blocks of row order[4g+b]; s = rhs strip.
    # n == -1 is a zero-weight dummy for otherwise-untouched PSUM regions.
    slots = [[[] for _ in range(4)] for _ in range(NG)]
    for n in range(NNZ):
        g, b = place[int(rows[n])]
        slots[g][b].append([int(cols[n]) // 4, int(cols[n]) % 4, n])
    for r in range(NRB):
        if cnt[r] == 0:
            g, b = place[r]
            slots[g][b].append([0, 0, -1])
    for g in range(NG):
        for b in range(4):
            slots[g][b].sort(key=lambda s: s[0])

    n_gb = np.array([[len(slots[g][b]) for b in range(4)] for g in range(NG)])

    MAX_ACTIVE = 7
    remaining = [[list(l) for l in gl] for gl in slots]
    done_g = [all(not l for l in gl) for gl in remaining]
    seen_gb = np.zeros((NG, 4), dtype=np.int64)
    waves = []
    scheduled = 0
    total = int(n_gb.sum())
    while scheduled < total:
        wv = len(waves)
        maxchunk = 1 + wv // 20
        used_a = set()
        wave = []
        acts = [g for g in range(NG) if not done_g[g]][:MAX_ACTIVE]
        for b in range(4):
            pick = None
            for g in acts:
                if (b + g) % 4 in used_a:
                    continue
                for i, (t, s, n) in enumerate(remaining[g][b]):
                    if t // XC >= maxchunk:
                        continue
                    pick = (g, i, t, s, n)
                    break
                if pick:
                    break
            if pick is None:
                continue
            g, i, t, s, n = pick
            remaining[g][b].pop(i)
            used_a.add((b + g) % 4)
            st = seen_gb[g, b] == 0
            sp = seen_gb[g, b] == n_gb[g, b] - 1
            seen_gb[g, b] += 1
            wave.append((s, g, b, t, n, bool(st), bool(sp)))
            scheduled += 1
            if all(not l for l in remaining[g]):
                done_g[g] = True
        waves.append(wave)

    W = len(waves)
    WCH = -(-W // NWCH)
    Wpad = WCH * NWCH
    wpk = np.zeros((128, Wpad * BLK), dtype=ml_dtypes.bfloat16)
    for wv, wave in enumerate(waves):
        for s, g, b, t, n, st, sp in wave:
            if n >= 0:
                A = (b + g) % 4
                wpk[32 * A:32 * A + 32, wv * BLK:(wv + 1) * BLK] = \
                    np.ascontiguousarray(w[n].T).astype(ml_dtypes.bfloat16)

    # wave index after which each group is fully accumulated
    evac = [-1] * NG
    for wv, wave in enumerate(waves):
        for s, g, b, t, n, st, sp in wave:
            evac[g] = max(evac[g], wv)
    return waves, WCH, Wpad, wpk, order, evac


def _build_module(waves, WCH, Wpad, evac):
    import concourse.bacc as bacc
    import concourse.tile as tile
    import concourse.mybir as mybir
    from contextlib import ExitStack

    f32 = mybir.dt.float32
    bf16 = mybir.dt.bfloat16
    nc = bacc.Bacc()
    xt_d = nc.declare_dram_parameter("xt", [128, 4, NT * BC], bf16,
                                     isOutput=False)
    wp_d = nc.declare_dram_parameter("wpk", [128, Wpad * BLK], bf16,
                                     isOutput=False)
    yt_d = nc.declare_dram_parameter("yt", [128, NG, BC], f32, isOutput=True)

    with tile.TileContext(nc) as tc, ExitStack() as ctx:
        xp = ctx.enter_context(tc.tile_pool(name="x", bufs=1))
        wpool = ctx.enter_context(tc.tile_pool(name="w", bufs=3))
        pp = ctx.enter_context(tc.tile_pool(name="ps", bufs=8, space="PSUM"))
        yp = ctx.enter_context(tc.tile_pool(name="y", bufs=4))

        wtiles = {}

        def load_w(c):
            wsb = wpool.tile([128, WCH * BLK], bf16, tag="w", name=f"w{c}")
            nc.sync.dma_start(
                wsb[:], wp_d[:, c * WCH * BLK:(c + 1) * WCH * BLK])
            wtiles[c] = wsb

        # xrot[k][ci]: x^T chunk pre-rotated on host so that original
        # strip s sits at partition strip (s + k) % 4.  Any column can feed
        # any PE row group while weights/fmap/tile_position stay aligned.
        # DMA issue costs ~600ns of sequencer time apiece, so issues are
        # split across the two HWDGE sequencers (sync + scalar).
        xrot = [[] for _ in range(4)]

        def load_x(ci):
            lo, hi = ci * XC * BC, (ci + 1) * XC * BC
            for k in range(4):
                xc = xp.tile([128, XC * BC], bf16, tag=f"x{k}_{ci}",
                             name=f"x{k}_{ci}")
                eng = nc.sync if k < 2 else nc.scalar
                eng.dma_start(xc[:], xt_d[:, k, lo:hi])
                xrot[k].append(xc)

        load_w(0)
        for ci in range(3):
            load_x(ci)
        load_w(1)

        ptiles = {}
        for wv, wave in enumerate(waves):
            c = wv // WCH
            if wv == c * WCH and c + 2 < NWCH and (c + 2) * WCH < len(waves):
                load_w(c + 2)
            if wv % 20 == 0 and wv // 20 + 3 < NXCH:
                load_x(wv // 20 + 3)
            wsb = wtiles[c]
            for s, g, b, t, n, st, sp in wave:
                if g not in ptiles:
                    ptiles[g] = pp.tile([128, BC], f32, tag="ps",
                                        name=f"ps{g}")
                A = (b + g) % 4
                nc.tensor.matmul(
                    ptiles[g][32 * b:32 * b + 32, :],
                    lhsT=wsb[32 * A:32 * A + 32,
                             (wv - c * WCH) * BLK:(wv - c * WCH + 1) * BLK],
                    rhs=xrot[(A - s) % 4][t // XC][
                        32 * A:32 * A + 32,
                        (t % XC) * BC:(t % XC + 1) * BC],
                    start=st, stop=sp, skip_group_check=True,
                    tile_position=(32 * A, 32 * b),
                )
            for g in range(NG):
                if evac[g] == wv:
                    ps = ptiles.pop(g)
                    ysb = yp.tile([128, BC], f32, tag="y", name=f"y{g}")
                    nc.vector.tensor_copy(ysb[:], ps[:])
                    nc.scalar.dma_start(yt_d[:, g, :], ysb[:])

    nc.compile()
    return nc


def kernel(x, w, rows, cols, out_blocks=None):
    import ml_dtypes
    from concourse.bass_utils import run_bass_kernel_spmd

    x = np.asarray(x, dtype=np.float32)
    w = np.asarray(w, dtype=np.float32)
    rows = np.asarray(rows).astype(np.int64)
    cols = np.asarray(cols).astype(np.int64)

    waves, WCH, Wpad, wpk, order, evac = _build_schedule(w, rows, cols)
    nc = _build_module(waves, WCH, Wpad, evac)

    # x^T, per-core partition-major: xarr[core, p, t*BC + j] = x[BC*core + j, 128*t + p]
    xarr = np.ascontiguousarray(
        x.reshape(NCORES, BC, NT, 128).transpose(0, 3, 2, 1)
    ).reshape(NCORES, 128, NT * BC).astype(ml_dtypes.bfloat16)
    # 4 partition rotations: xrot[core, p, k, :] = xarr[core, (p - 32k) % 128, :]
    xrot = np.stack([np.roll(xarr, 32 * k, axis=1) for k in range(4)],
                    axis=2)

    in_maps = [{"xt": xrot[i], "wpk": wpk} for i in range(NCORES)]
    res = run_bass_kernel_spmd(
        nc, in_maps, list(range(NCORES)),
        trace=_RUN["trace"], trace_cores=_RUN["trace_cores"],
    )
    _RUN["last"] = res

    # feature index of flat position (k=4g+b, i): 32*order[k] + i
    feat = (32 * order[:, None] + np.arange(32)[None, :]).ravel()

    y = np.empty((B, OUT), dtype=np.float32)
    for i in range(NCORES):
        ytp = np.asarray(res.results[i]["yt"]).astype(np.float32)
        # [128, NG, 256]: partition 32b+i, group g, batch j -> k=4g+b
        v = ytp.reshape(4, 32, NG, BC).transpose(2, 0, 1, 3)
        yT = np.empty((OUT, BC), dtype=np.float32)
        yT[feat] = v.reshape(OUT, BC)
        y[BC * i:BC * (i + 1), :] = yT.T
    return y
